# revision 1
# baseline (speedup 1.0000x reference)
"""3-layer GCN (message passing) on 8 NeuronCores via Bass/Tile.

Strategy (vertex-cut / dst-sharding):
  - Node i's output row is owned by core i // 6250.
  - out = relu(A_hat @ (X W) + b) per layer, A_hat = D^-1/2 (A+I) D^-1/2.
  - Fold dinv[src] into the gather table (Y = dinv * (X W)); fold dinv[dst]
    into host-built one-hot scatter matrices S (S[e, dst_rel] = dinv[dst]).
  - Scatter-add realized as TensorE matmuls: psum[dst,feat] += S^T @ Y[src].
  - Layer 1 table computed redundantly per core from the replicated input X.
    Layers 2/3: each core computes its Y shard, AllGather to a full table.
"""

import os
import sys

sys.path.insert(0, "/opt/trn_rl_repo")

import numpy as np

N = 50000
E = 500000
NC = 8
SH = N // NC            # 6250 nodes per core
P = 128
DIN = 128
DH = 256
NBLK = (SH + P - 1) // P  # 49 dst blocks per core; last block has 106 nodes
LASTM = SH - (NBLK - 1) * P  # 106
NW = (N + P - 1) // P   # 391 windows over all nodes; last has 80
LASTW = N - (NW - 1) * P  # 80


def _preprocess(x, edge_index):
    """Host-side graph partitioning. Returns per-core tensors + layout meta."""
    src = np.asarray(edge_index[0], dtype=np.int64)
    dst = np.asarray(edge_index[1], dtype=np.int64)
    deg = np.bincount(dst, minlength=N).astype(np.float64) + 1.0
    dinv = (1.0 / np.sqrt(deg)).astype(np.float32)

    order = np.argsort(dst, kind="stable")
    s_s = src[order]
    d_s = dst[order]

    # per-core edge ranges (d_s sorted ascending)
    bounds = np.searchsorted(d_s, np.arange(NC + 1) * SH)

    # chunk counts per (core, block)
    cnts = np.zeros((NC, NBLK), dtype=np.int64)
    for c in range(NC):
        lo, hi = bounds[c], bounds[c + 1]
        blk = (d_s[lo:hi] - c * SH) >> 7
        cnts[c] = np.bincount(blk, minlength=NBLK)
    cpb = np.maximum(1, (cnts.max(axis=0) + P - 1) // P)  # chunks per block
    cum = np.concatenate([[0], np.cumsum(cpb)])           # edge-chunk offsets
    nch = int(cum[-1])
    # gather-column offsets: per block cp edge chunks + 1 self chunk
    gcum = cum[:-1] + np.arange(NBLK)
    ngc = nch + NBLK

    per_core = []
    for c in range(NC):
        lo, hi = bounds[c], bounds[c + 1]
        sc = s_s[lo:hi]
        dc = (d_s[lo:hi] - c * SH).astype(np.int64)
        blk = dc >> 7
        n_e = hi - lo
        starts = np.concatenate([[0], np.cumsum(cnts[c])])
        pos = np.arange(n_e) - starts[blk]
        chunk = cum[blk] + (pos >> 7)
        prow = pos & 127

        idx_all = np.zeros((P, ngc), dtype=np.int32)
        s_all = np.zeros((P, nch * P), dtype=np.float32)
        gcol = gcum[blk] + (pos >> 7)
        idx_all[prow, gcol] = sc.astype(np.int32)
        s_all[prow, chunk * P + (dc & 127)] = dinv[dc + c * SH]

        # self columns: block b -> gather column gcum[b] + cpb[b]
        ids = c * SH + np.arange(NBLK * P)
        valid = ids < (c + 1) * SH
        ids_c = np.where(valid, ids, 0).reshape(NBLK, P).T.astype(np.int32)
        selfcol = (gcum + cpb).astype(np.int64)
        idx_all[:, selfcol] = ids_c

        dcol = np.where(valid, dinv[np.minimum(ids, N - 1)], 0.0)
        dcol = dcol.reshape(NBLK, P).T.astype(np.float32)
        per_core.append({"idx": idx_all, "sall": s_all, "dcol": dcol})

    # dinv for all nodes, [P, NW] layout (window-major)
    dpad = np.zeros(NW * P, dtype=np.float32)
    dpad[:N] = dinv
    dfull = dpad.reshape(NW, P).T.copy()

    meta = {"cpb": cpb.tolist(), "cum": cum.tolist(), "gcum": gcum.tolist(),
            "nch": nch, "ngc": ngc}
    return per_core, dfull, meta


def _build_program(meta):
    from concourse import bass, bacc, mybir
    import concourse.tile as tile
    from concourse.masks import make_identity

    f32 = mybir.dt.float32
    i32 = mybir.dt.int32
    cpb, gcum, cum = meta["cpb"], meta["gcum"], meta["cum"]
    nch, ngc = meta["nch"], meta["ngc"]

    nc = bacc.Bacc("TRN2", target_bir_lowering=False, debug=False)

    xt = nc.declare_dram_parameter("xt", [P, N], f32, isOutput=False)
    w1 = nc.declare_dram_parameter("w1", [DIN, DH], f32, isOutput=False)
    w2 = nc.declare_dram_parameter("w2", [DH, DH], f32, isOutput=False)
    w3 = nc.declare_dram_parameter("w3", [DH, DH], f32, isOutput=False)
    bf1 = nc.declare_dram_parameter("bf1", [P, DH], f32, isOutput=False)
    bf2 = nc.declare_dram_parameter("bf2", [P, DH], f32, isOutput=False)
    bf3 = nc.declare_dram_parameter("bf3", [P, DH], f32, isOutput=False)
    idx = nc.declare_dram_parameter("idx", [P, ngc], i32, isOutput=False)
    sall = nc.declare_dram_parameter("sall", [P, nch * P], f32, isOutput=False)
    dcol = nc.declare_dram_parameter("dcol", [P, NBLK], f32, isOutput=False)
    dful = nc.declare_dram_parameter("dful", [P, NW], f32, isOutput=False)
    outp = nc.declare_dram_parameter("out", [SH, DH], f32, isOutput=True)

    # internal DRAM
    y1 = nc.dram_tensor("y1", [N, DH], f32)           # L1 gather table (full, local)
    ybin2 = nc.dram_tensor("ybin2", [SH, DH], f32)
    ybout2 = nc.dram_tensor("ybout2", [N, DH], f32, addr_space="Shared")
    ybin3 = nc.dram_tensor("ybin3", [SH, DH], f32)
    ybout3 = nc.dram_tensor("ybout3", [N, DH], f32, addr_space="Shared")

    WIN_PER_XBIG = 16

    with tile.TileContext(nc, linearize=bool(__import__("os").environ.get("KLIN"))) as tc:
        with (
            tc.tile_pool(name="const", bufs=1) as cp_,
            tc.tile_pool(name="sb", bufs=3) as sb,
            tc.tile_pool(name="gp", bufs=2) as gp,
            tc.tile_pool(name="xb", bufs=2) as xbp,
            tc.tile_pool(name="pp", bufs=2, space="PSUM") as pp,
        ):
            ident = cp_.tile([P, P], dtype=f32)
            make_identity(nc, ident[:])
            w1sb = cp_.tile([P, DH], dtype=f32)
            nc.sync.dma_start(out=w1sb[:], in_=w1[:, :])
            w2sb = cp_.tile([P, 2 * DH], dtype=f32)
            w3sb = cp_.tile([P, 2 * DH], dtype=f32)
            for k in range(2):
                nc.sync.dma_start(out=w2sb[:, k * DH:(k + 1) * DH],
                                  in_=w2[k * P:(k + 1) * P, :])
                nc.sync.dma_start(out=w3sb[:, k * DH:(k + 1) * DH],
                                  in_=w3[k * P:(k + 1) * P, :])
            bsb = []
            for bt in (bf1, bf2, bf3):
                t = cp_.tile([P, DH], dtype=f32, tag=f"b_{bt.name}")
                nc.sync.dma_start(out=t[:], in_=bt[:, :])
                bsb.append(t)
            idxsb = cp_.tile([P, ngc], dtype=i32)
            nc.sync.dma_start(out=idxsb[:], in_=idx[:, :])
            dcolsb = cp_.tile([P, NBLK], dtype=f32)
            nc.sync.dma_start(out=dcolsb[:], in_=dcol[:, :])
            dfulsb = cp_.tile([P, NW], dtype=f32)
            nc.sync.dma_start(out=dfulsb[:], in_=dful[:, :])
            # resident transposed activations for layers 2/3: [feat, 2*SH]
            xts = cp_.tile([P, 2 * SH], dtype=f32)

            AG = mybir.AluOpType
            ACT = mybir.ActivationFunctionType

            # ---------------- Layer 1 phase 1: full Y1 (redundant) ----------
            for t in range((NW + WIN_PER_XBIG - 1) // WIN_PER_XBIG):
                wlo = t * WIN_PER_XBIG
                whi = min(wlo + WIN_PER_XBIG, NW)
                ncols = min(whi * P, N) - wlo * P
                xbig = xbp.tile([P, WIN_PER_XBIG * P], dtype=f32, tag="xbig")
                nc.sync.dma_start(out=xbig[:, :ncols],
                                  in_=xt[:, wlo * P: wlo * P + ncols])
                for w in range(wlo, whi):
                    m = min(P, N - w * P)
                    ps = pp.tile([P, DH], dtype=f32, tag="hps")
                    nc.tensor.matmul(
                        out=ps[:m, :],
                        lhsT=xbig[:, (w - wlo) * P:(w - wlo) * P + m],
                        rhs=w1sb[:], start=True, stop=True)
                    ysb = sb.tile([P, DH], dtype=f32, tag="ysb")
                    nc.scalar.activation(out=ysb[:m, :], in_=ps[:m, :],
                                         func=ACT.Copy,
                                         scale=dfulsb[:m, w:w + 1])
                    nc.sync.dma_start(out=y1[w * P: w * P + m, :],
                                      in_=ysb[:m, :])

            def scatter_layer(l, table, b_tile, next_phase):
                """Scatter phase of layer l reading from full table."""
                for b in range(NBLK):
                    cp = cpb[b]
                    goff = gcum[b]
                    m = LASTM if b == NBLK - 1 else P
                    gt = gp.tile([P, (max(cpb) + 1) * DH], dtype=f32, tag="gt")
                    # HW DGE honors ONE index per partition per indirect DMA
                    # (extra offset columns are ignored; payload is read
                    # contiguously) -> one gather per 128-edge chunk.
                    for k in range(cp + 1):
                        nc.gpsimd.indirect_dma_start(
                            out=gt[:, k * DH:(k + 1) * DH], out_offset=None,
                            in_=table[:, :],
                            in_offset=bass.IndirectOffsetOnAxis(
                                ap=idxsb[:, goff + k:goff + k + 1], axis=0))
                    st = sb.tile([P, max(cpb) * P], dtype=f32, tag="st")
                    nc.sync.dma_start(
                        out=st[:, :cp * P],
                        in_=sall[:, cum[b] * P:(cum[b] + cp) * P])
                    ps = pp.tile([P, DH], dtype=f32, tag="agg")
                    for k in range(cp):
                        nc.tensor.matmul(out=ps[:], lhsT=st[:, k * P:(k + 1) * P],
                                         rhs=gt[:, k * DH:(k + 1) * DH],
                                         start=(k == 0), stop=(k == cp - 1))
                    # epilogue: out = [relu](psum + dinv*Y_self + b)
                    tmp = sb.tile([P, DH], dtype=f32, tag="tmp")
                    nc.vector.tensor_tensor(
                        out=tmp[:], in0=gt[:, cp * DH:(cp + 1) * DH],
                        in1=dcolsb[:, b:b + 1].to_broadcast([P, DH]),
                        op=AG.mult)
                    nc.vector.tensor_tensor(out=tmp[:], in0=tmp[:],
                                            in1=b_tile[:], op=AG.add)
                    if l < 3:
                        tmp2 = sb.tile([P, DH], dtype=f32, tag="tmp2")
                        nc.vector.tensor_tensor(out=tmp2[:], in0=tmp[:],
                                                in1=ps[:], op=AG.add)
                        xn = sb.tile([P, DH], dtype=f32, tag="xn")
                        nc.scalar.activation(out=xn[:], in_=tmp2[:],
                                             func=ACT.Relu)
                        for k in range(2):
                            tps = pp.tile([P, P], dtype=f32, tag="tp")
                            nc.tensor.transpose(
                                out=tps[:, :m],
                                in_=xn[:m, k * P:(k + 1) * P],
                                identity=ident[:m, :m])
                            nc.vector.tensor_copy(
                                out=xts[:, k * SH + b * P: k * SH + b * P + m],
                                in_=tps[:, :m])
                    else:
                        osb = sb.tile([P, DH], dtype=f32, tag="osb")
                        nc.vector.tensor_tensor(out=osb[:], in0=tmp[:],
                                                in1=ps[:], op=AG.add)
                        nc.sync.dma_start(out=outp[b * P: b * P + m, :],
                                          in_=osb[:m, :])
                if next_phase is not None:
                    next_phase()

            def phase1(wsb, ybin, ybout):
                """H = X@W for own shard from xts; write Y shard; AllGather."""
                for w in range(NBLK):
                    m = LASTM if w == NBLK - 1 else P
                    ps = pp.tile([P, DH], dtype=f32, tag="hps")
                    for k in range(2):
                        nc.tensor.matmul(
                            out=ps[:m, :],
                            lhsT=xts[:, k * SH + w * P: k * SH + w * P + m],
                            rhs=wsb[:, k * DH:(k + 1) * DH],
                            start=(k == 0), stop=(k == 1))
                    ysb = sb.tile([P, DH], dtype=f32, tag="ysb")
                    nc.scalar.activation(out=ysb[:m, :], in_=ps[:m, :],
                                         func=ACT.Copy,
                                         scale=dcolsb[:m, w:w + 1])
                    nc.sync.dma_start(out=ybin[w * P: w * P + m, :],
                                      in_=ysb[:m, :])
                nc.gpsimd.collective_compute(
                    "AllGather", AG.bypass,
                    replica_groups=[list(range(NC))],
                    ins=[ybin.ap().opt()],
                    outs=[ybout.ap().opt()])

            scatter_layer(1, y1, bsb[0],
                          lambda: phase1(w2sb, ybin2, ybout2))
            scatter_layer(2, ybout2, bsb[1],
                          lambda: phase1(w3sb, ybin3, ybout3))
            scatter_layer(3, ybout3, bsb[2], None)

            dbg = os.environ.get("KDBG")
            if dbg:
                src_t = {"y1": y1, "yb2": ybout2, "ybin2": ybin2,
                         "yb3": ybout3}[dbg]
                nc.sync.dma_start(out=outp[:, :], in_=src_t[0:SH, :])

    nc.compile()
    return nc


_CACHED = None


def _get_program_and_data(x, edge_index):
    global _CACHED
    per_core, dfull, meta = _preprocess(x, edge_index)
    nc = _build_program(meta)
    return nc, per_core, dfull


def kernel(x, edge_index, W1, b1, W2, b2, W3, b3, _trace=False):
    from concourse.bass_utils import run_bass_kernel_spmd

    x = np.asarray(x, dtype=np.float32)
    nc, per_core, dfull = _get_program_and_data(x, edge_index)

    xt = np.ascontiguousarray(x.T)
    common = {
        "xt": xt,
        "w1": np.asarray(W1, dtype=np.float32),
        "w2": np.asarray(W2, dtype=np.float32),
        "w3": np.asarray(W3, dtype=np.float32),
        "bf1": np.broadcast_to(np.asarray(b1, np.float32), (P, DH)).copy(),
        "bf2": np.broadcast_to(np.asarray(b2, np.float32), (P, DH)).copy(),
        "bf3": np.broadcast_to(np.asarray(b3, np.float32), (P, DH)).copy(),
        "dful": dfull,
    }
    in_maps = []
    for c in range(NC):
        m = dict(common)
        m["idx"] = per_core[c]["idx"]
        m["sall"] = per_core[c]["sall"]
        m["dcol"] = per_core[c]["dcol"]
        in_maps.append(m)

    res = run_bass_kernel_spmd(nc, in_maps, list(range(NC)), trace=_trace)
    shards = [res.results[c]["out"] for c in range(NC)]
    out = np.concatenate(shards, axis=0)
    if _trace:
        return out, res
    return out



# revision 4
# speedup vs baseline: 1.4319x; 1.4319x over previous
"""3-layer GCN (message passing) on 8 NeuronCores via Bass/Tile.

Strategy (vertex-cut / dst-sharding), v2:
  - Node i's output row is owned by core i // 6250.
  - Per layer: Y = dinv * (X @ W) (per-node rows, bf16) computed for the
    own shard, AllGathered to a full gather table [N, 256] bf16.
  - Scatter-add realized on TensorE: psum[dst,feat] += S^T @ Y[src] where
    S is a one-hot fp8 edge->dst matrix (dinv[dst] folded into the ACT
    epilogue scale, so S entries are exactly 1.0).
  - Self loop: identity-matmul from the SBUF-resident own-shard Y tile
    (no DMA), exact since table rows already carry dinv[src].
  - Gathers: batched SWDGE dma_gather (int16 indices). Node ids don't fit
    int16, so the table is viewed as [N/2, 512] row-pairs and edges are
    split by src parity: even srcs gather from columns [0,256), odd srcs
    from [256,512), each with idx = src >> 1 <= 25000.
"""

import numpy as np

import sys

sys.path.insert(0, "/opt/trn_rl_repo")

N = 50000
E = 500000
NC = 8
SH = N // NC              # 6250 nodes per core
P = 128
DIN = 128
DH = 256
NBLK = (SH + P - 1) // P  # 49 dst blocks per core
LASTM = SH - (NBLK - 1) * P  # 106


def _preprocess(edge_index):
    """Host-side graph partitioning. Returns per-core tensors + layout."""
    import ml_dtypes

    src = np.asarray(edge_index[0], dtype=np.int64)
    dst = np.asarray(edge_index[1], dtype=np.int64)
    deg = np.bincount(dst, minlength=N).astype(np.float64) + 1.0
    dinv = (1.0 / np.sqrt(deg)).astype(np.float32)

    order = np.argsort(dst, kind="stable")
    s_s = src[order]
    d_s = dst[order]
    bounds = np.searchsorted(d_s, np.arange(NC + 1) * SH)

    # per-core (block, parity) grouping
    cores = []
    cnts = np.zeros((NC, 2 * NBLK), dtype=np.int64)
    for c in range(NC):
        lo, hi = bounds[c], bounds[c + 1]
        sc = s_s[lo:hi]
        dc = (d_s[lo:hi] - c * SH).astype(np.int64)
        key = (dc >> 7) * 2 + (sc & 1)
        o2 = np.argsort(key, kind="stable")
        cores.append((sc[o2], dc[o2], key[o2]))
        cnts[c] = np.bincount(key, minlength=2 * NBLK)

    mx = cnts.max(axis=0)
    cpe = np.maximum(1, (mx[0::2] + P - 1) // P)   # even chunks per block
    cpo = np.maximum(1, (mx[1::2] + P - 1) // P)   # odd chunks per block
    cpt = cpe + cpo
    scum = np.concatenate([[0], np.cumsum(cpt)])   # chunk offset per block
    nch = int(scum[-1])

    per_core = []
    for c in range(NC):
        sc2, dc2, key2 = cores[c]
        n_e = sc2.size
        starts = np.concatenate([[0], np.cumsum(cnts[c])])
        pos = np.arange(n_e) - starts[key2]
        kb = key2 >> 1
        kp = key2 & 1
        cbase = scum[kb] + np.where(kp == 1, cpe[kb], 0)
        chunk = cbase + (pos >> 7)
        prow = pos & 127

        s_f32 = np.zeros((P, nch * P), dtype=np.float32)
        s_f32[prow, chunk * P + (dc2 & 127)] = 1.0
        sall = s_f32.astype(ml_dtypes.float8_e4m3)

        idxflat = np.zeros(nch * P, dtype=np.int16)
        idxflat[chunk * P + prow] = (sc2 >> 1).astype(np.int16)
        idx16 = np.ascontiguousarray(
            np.tile(idxflat.reshape(nch * 8, 16).T, (8, 1))
        )

        ids = c * SH + np.minimum(np.arange(NBLK * P), SH - 1)
        dcol = dinv[ids].reshape(NBLK, P).T.copy()  # [P, NBLK]
        per_core.append({"idx": idx16, "sall": sall, "dcol": dcol})

    meta = {"cpe": cpe.tolist(), "cpo": cpo.tolist(),
            "scum": scum.tolist(), "nch": nch}
    return per_core, dinv, meta


def _build_program(meta, with_bias):
    from concourse import bass, bacc, mybir
    import concourse.tile as tile
    from concourse import library_config

    f32 = mybir.dt.float32
    bf16 = mybir.dt.bfloat16
    f8 = mybir.dt.float8e4
    i16 = mybir.dt.int16
    AG = mybir.AluOpType
    ACT = mybir.ActivationFunctionType

    cpe, cpo, scum, nch = meta["cpe"], meta["cpo"], meta["scum"], meta["nch"]
    NB2 = NBLK
    CPT_MAX = max(cpe[b] + cpo[b] for b in range(NB2))

    nc = bacc.Bacc("TRN2", target_bir_lowering=False, debug=False)

    xtsh = nc.declare_dram_parameter("xtsh", [P, SH], bf16, isOutput=False)
    w1 = nc.declare_dram_parameter("w1", [P, DH], bf16, isOutput=False)
    w2f = nc.declare_dram_parameter("w2f", [P, 2 * DH], bf16, isOutput=False)
    w3f = nc.declare_dram_parameter("w3f", [P, 2 * DH], bf16, isOutput=False)
    idx = nc.declare_dram_parameter("idx", [P, nch * 8], i16, isOutput=False)
    sall = nc.declare_dram_parameter("sall", [P, nch * P], f8, isOutput=False)
    dcol = nc.declare_dram_parameter("dcol", [P, NBLK], f32, isOutput=False)
    id8 = nc.declare_dram_parameter("id8", [P, P], f8, isOutput=False)
    idb = nc.declare_dram_parameter("idb", [P, P], bf16, isOutput=False)
    if with_bias:
        badj = [nc.declare_dram_parameter(f"badj{l}", [SH, DH], f32,
                                          isOutput=False) for l in (1, 2, 3)]
    outp = nc.declare_dram_parameter("out", [SH, DH], f32, isOutput=True)

    ybin = [nc.dram_tensor(f"ybin{l}", [SH, DH], bf16) for l in (1, 2, 3)]
    ybout = [nc.dram_tensor(f"ybout{l}", [N // 2, 2 * DH], bf16,
                            addr_space="Shared") for l in (1, 2, 3)]

    with tile.TileContext(nc) as tc:
        with (
            tc.tile_pool(name="const", bufs=1) as cp_,
            tc.tile_pool(name="sb", bufs=3) as sb,
            tc.tile_pool(name="gp", bufs=2) as gp,
            tc.tile_pool(name="stp", bufs=2) as stp,
            tc.tile_pool(name="pp", bufs=2, space="PSUM") as pp,
            tc.tile_pool(name="tp", bufs=2, space="PSUM") as tpp,
        ):
            # library for dma_gather; must precede all gathers on GpSimd
            nc.gpsimd.load_library(library_config.mlp)

            w1sb = cp_.tile([P, DH], dtype=bf16)
            nc.sync.dma_start(out=w1sb[:], in_=w1[:, :])
            w2sb = cp_.tile([P, 2 * DH], dtype=bf16)
            nc.sync.dma_start(out=w2sb[:], in_=w2f[:, :])
            w3sb = cp_.tile([P, 2 * DH], dtype=bf16)
            nc.sync.dma_start(out=w3sb[:], in_=w3f[:, :])
            idxsb = cp_.tile([P, nch * 8], dtype=i16)
            nc.sync.dma_start(out=idxsb[:], in_=idx[:, :])
            dcolsb = cp_.tile([P, NBLK], dtype=f32)
            nc.sync.dma_start(out=dcolsb[:], in_=dcol[:, :])
            id8sb = cp_.tile([P, P], dtype=f8)
            nc.sync.dma_start(out=id8sb[:], in_=id8[:, :])
            idbsb = cp_.tile([P, P], dtype=bf16)
            nc.sync.dma_start(out=idbsb[:], in_=idb[:, :])
            xts1 = cp_.tile([P, SH], dtype=bf16)
            nc.sync.dma_start(out=xts1[:], in_=xtsh[:, :])
            xts = cp_.tile([P, 2 * SH], dtype=bf16)   # layers 2/3 lhsT
            yA = cp_.tile([P, NBLK * DH], dtype=bf16)  # own Y, layers 1/3
            yB = cp_.tile([P, NBLK * DH], dtype=bf16)  # own Y, layer 2

            def phase1(w, lhs_slices, wsb, yown, yb):
                m = LASTM if w == NBLK - 1 else P
                ps = pp.tile([P, DH], dtype=f32, tag="p1ps")
                nk = len(lhs_slices)
                for k, lhs in enumerate(lhs_slices):
                    nc.tensor.matmul(out=ps[:m, :], lhsT=lhs,
                                     rhs=wsb[:, k * DH:(k + 1) * DH],
                                     start=(k == 0), stop=(k == nk - 1))
                nc.scalar.activation(out=yown[:m, w * DH:(w + 1) * DH],
                                     in_=ps[:m, :], func=ACT.Copy,
                                     scale=dcolsb[:m, w:w + 1])
                nc.sync.dma_start(out=yb[w * P:w * P + m, :],
                                  in_=yown[:m, w * DH:(w + 1) * DH])

            def allgather(l):
                nc.gpsimd.collective_compute(
                    "AllGather", AG.bypass,
                    replica_groups=[list(range(NC))],
                    ins=[ybin[l].ap().opt()],
                    outs=[ybout[l].ap().opt()])

            # ---- layer-1 phase1: Y1 = dinv * (X @ W1), own shard ----
            for w in range(NBLK):
                m = LASTM if w == NBLK - 1 else P
                phase1(w, [xts1[:, w * P:w * P + m]], w1sb, yA, ybin[0])
            allgather(0)

            def scatter(l, table, yown_r, yown_w, yb_w, wnext):
                """Scatter+aggregate layer l (1-based); inline phase1 of
                layer l+1 per block when l < 3."""
                for b in range(NBLK):
                    m = LASTM if b == NBLK - 1 else P
                    ce, co = cpe[b], cpo[b]
                    ct = ce + co
                    ic = scum[b] * 8
                    gt = gp.tile([P, CPT_MAX, DH], dtype=bf16, tag="gt")
                    nc.gpsimd.dma_gather(
                        gt[:, 0:ce, :], table[:, 0:DH],
                        idxsb[:, ic:ic + ce * 8],
                        ce * P, ce * P, DH, elem_step=2 * DH, queue_num=0)
                    nc.gpsimd.dma_gather(
                        gt[:, ce:ct, :], table[:, DH:2 * DH],
                        idxsb[:, ic + ce * 8:ic + ct * 8],
                        co * P, co * P, DH, elem_step=2 * DH, queue_num=0)
                    stt = stp.tile([P, CPT_MAX * P], dtype=f8, tag="stt")
                    nc.sync.dma_start(
                        out=stt[:, :ct * P],
                        in_=sall[:, scum[b] * P:(scum[b] + ct) * P])
                    ps = pp.tile([P, DH], dtype=f32, tag="agg")
                    for k in range(ct):
                        nc.tensor.matmul(out=ps[:],
                                         lhsT=stt[:, k * P:(k + 1) * P],
                                         rhs=gt[:, k, :],
                                         start=(k == 0), stop=False)
                    nc.tensor.matmul(out=ps[:], lhsT=id8sb[:],
                                     rhs=yown_r[:, b * DH:(b + 1) * DH],
                                     start=False, stop=True)
                    if with_bias:
                        bj = sb.tile([P, DH], dtype=f32, tag="bj")
                        nc.sync.dma_start(out=bj[:m, :],
                                          in_=badj[l - 1][b * P:b * P + m, :])
                        nc.vector.tensor_tensor(out=ps[:m, :], in0=ps[:m, :],
                                                in1=bj[:m, :], op=AG.add)
                    if l < 3:
                        xn = sb.tile([P, DH], dtype=bf16, tag="xn")
                        nc.scalar.activation(out=xn[:m, :], in_=ps[:m, :],
                                             func=ACT.Relu,
                                             scale=dcolsb[:m, b:b + 1])
                        for k in range(2):
                            tps = tpp.tile([P, P], dtype=bf16, tag="tps")
                            nc.tensor.transpose(
                                out=tps[:, :m],
                                in_=xn[:m, k * P:(k + 1) * P],
                                identity=idbsb[:m, :m])
                            nc.vector.tensor_copy(
                                out=xts[:, k * SH + b * P:k * SH + b * P + m],
                                in_=tps[:, :m])
                        phase1(b, [xts[:, b * P:b * P + m],
                                   xts[:, SH + b * P:SH + b * P + m]],
                               wnext, yown_w, yb_w)
                    else:
                        osb = sb.tile([P, DH], dtype=f32, tag="osb")
                        nc.scalar.activation(out=osb[:m, :], in_=ps[:m, :],
                                             func=ACT.Copy,
                                             scale=dcolsb[:m, b:b + 1])
                        nc.sync.dma_start(out=outp[b * P:b * P + m, :],
                                          in_=osb[:m, :])
                if l < 3:
                    allgather(l)

            scatter(1, ybout[0], yA, yB, ybin[1], w2sb)
            scatter(2, ybout[1], yB, yA, ybin[2], w3sb)
            scatter(3, ybout[2], yA, None, None, None)

    nc.compile()

    # sanity: library load must precede the first dma_gather on GpSimd
    pos_lib = pos_gather = None
    i = 0
    for blk in nc.m.functions[0].blocks:
        for inst in blk.instructions:
            tn = type(inst).__name__
            if pos_lib is None and tn == "InstPseudoReloadLibraryIndex":
                pos_lib = i
            if pos_gather is None and tn == "InstDMAGatherAnt":
                pos_gather = i
            i += 1
    assert pos_lib is not None, "library load missing"
    assert pos_gather is None or pos_lib < pos_gather, \
        f"library load at {pos_lib} after first gather at {pos_gather}"
    return nc


def kernel(x, edge_index, W1, b1, W2, b2, W3, b3, _trace=False):
    import ml_dtypes
    from concourse.bass_utils import run_bass_kernel_spmd

    bf16 = ml_dtypes.bfloat16
    f8 = ml_dtypes.float8_e4m3

    x = np.asarray(x, dtype=np.float32)
    b1 = np.asarray(b1, dtype=np.float32)
    b2 = np.asarray(b2, dtype=np.float32)
    b3 = np.asarray(b3, dtype=np.float32)
    with_bias = bool(np.any(b1) or np.any(b2) or np.any(b3))

    per_core, dinv, meta = _preprocess(edge_index)
    nc = _build_program(meta, with_bias)

    xt = np.ascontiguousarray(x.T)
    common = {
        "w1": np.asarray(W1, np.float32).astype(bf16),
        "w2f": np.ascontiguousarray(
            np.concatenate([np.asarray(W2, np.float32)[0:P, :],
                            np.asarray(W2, np.float32)[P:2 * P, :]], axis=1)
        ).astype(bf16),
        "w3f": np.ascontiguousarray(
            np.concatenate([np.asarray(W3, np.float32)[0:P, :],
                            np.asarray(W3, np.float32)[P:2 * P, :]], axis=1)
        ).astype(bf16),
        "id8": np.eye(P, dtype=np.float32).astype(f8),
        "idb": np.eye(P, dtype=np.float32).astype(bf16),
    }
    in_maps = []
    for c in range(NC):
        m = dict(common)
        m["xtsh"] = np.ascontiguousarray(
            xt[:, c * SH:(c + 1) * SH]).astype(bf16)
        m["idx"] = per_core[c]["idx"]
        m["sall"] = per_core[c]["sall"]
        m["dcol"] = per_core[c]["dcol"]
        if with_bias:
            dshard = dinv[c * SH:(c + 1) * SH]
            for l, b in ((1, b1), (2, b2), (3, b3)):
                m[f"badj{l}"] = np.ascontiguousarray(
                    b[None, :] / dshard[:, None]).astype(np.float32)
        in_maps.append(m)

    res = run_bass_kernel_spmd(nc, in_maps, list(range(NC)), trace=_trace)
    out = np.concatenate([res.results[c]["out"] for c in range(NC)], axis=0)
    if _trace:
        return out, res
    return out


# revision 6
# speedup vs baseline: 1.8780x; 1.3115x over previous
"""3-layer GCN (message passing) on 8 NeuronCores via Bass/Tile.

Strategy (vertex-cut / dst-sharding), v3:
  - Node i's output row is owned by core i // 6250.
  - Per layer: Y = dinv * (X @ W) (per-node rows, bf16) computed for the
    own shard; for layers 2/3 it is AllGathered to a full gather table
    [N, 256] bf16.
  - Scatter-add on TensorE: psum[dst,feat] += S^T @ Y[src] where S is a
    one-hot fp8 edge->dst matrix (dinv[dst] folded into the ACT epilogue
    scale, so S entries are exactly 1.0).
  - Self loop: identity-matmul from the SBUF-resident own-shard Y tile
    (exact: table rows already carry dinv[src]).
  - Layer 1 needs no gathers at all: the host stages edge-ordered rows
    Xg[e] = dinv[src_e] * X[src_e] (pure input reordering), and the
    aggregation commutes with the W1 matmul:
        agg_b = (sum_k Xg_k^T-contracted S_k)[din, dst]^T @ W1.
    One extra [128x128] matmul pair per block, zero DMA gathers, no AG1.
  - Layers 2/3 gathers: batched SWDGE dma_gather (int16 indices). Node
    ids don't fit int16, so the table is viewed as [N/2, 512] row-pairs
    and edges are split by src parity: even srcs gather from columns
    [0,256), odd from [256,512), idx = src >> 1 < 25000. Per-core
    shortfall vs the SPMD-uniform chunk count is padded with -1 (trailing
    only), which the Q7 descriptor generator skips.
"""

import numpy as np

import sys

sys.path.insert(0, "/opt/trn_rl_repo")

N = 50000
E = 500000
NC = 8
SH = N // NC              # 6250 nodes per core
P = 128
DIN = 128
DH = 256
NBLK = (SH + P - 1) // P  # 49 dst blocks per core
LASTM = SH - (NBLK - 1) * P  # 106


def _preprocess(x, edge_index):
    """Host-side graph partitioning. Returns per-core tensors + layout."""
    import ml_dtypes

    bf16 = ml_dtypes.bfloat16
    f8 = ml_dtypes.float8_e4m3

    x = np.asarray(x, dtype=np.float32)
    src = np.asarray(edge_index[0], dtype=np.int64)
    dst = np.asarray(edge_index[1], dtype=np.int64)
    deg = np.bincount(dst, minlength=N).astype(np.float64) + 1.0
    dinv = (1.0 / np.sqrt(deg)).astype(np.float32)
    xdi = x * dinv[:, None]          # dinv[src]-scaled input rows

    order = np.argsort(dst, kind="stable")
    s_s = src[order]
    d_s = dst[order]
    bounds = np.searchsorted(d_s, np.arange(NC + 1) * SH)

    cores = []
    cnts = np.zeros((NC, 2 * NBLK), dtype=np.int64)    # (block, parity)
    cnts1 = np.zeros((NC, NBLK), dtype=np.int64)       # block only (L1)
    for c in range(NC):
        lo, hi = bounds[c], bounds[c + 1]
        sc = s_s[lo:hi]
        dc = (d_s[lo:hi] - c * SH).astype(np.int64)
        key = (dc >> 7) * 2 + (sc & 1)
        o2 = np.argsort(key, kind="stable")
        cores.append((sc[o2], dc[o2], key[o2]))
        cnts[c] = np.bincount(key, minlength=2 * NBLK)
        cnts1[c] = np.bincount(dc >> 7, minlength=NBLK)

    # ---- layers 2/3 layout: (block, parity) chunks ----
    mx = cnts.max(axis=0)
    cpe = np.maximum(1, (mx[0::2] + P - 1) // P)
    cpo = np.maximum(1, (mx[1::2] + P - 1) // P)
    cpt = cpe + cpo
    scum = np.concatenate([[0], np.cumsum(cpt)])
    nch = int(scum[-1])

    # ---- layer-1 layout: per-block chunks, no parity ----
    cpb1 = np.maximum(1, (cnts1.max(axis=0) + P - 1) // P)
    scum1 = np.concatenate([[0], np.cumsum(cpb1)])
    nch1 = int(scum1[-1])

    per_core = []
    for c in range(NC):
        sc2, dc2, key2 = cores[c]
        n_e = sc2.size
        starts = np.concatenate([[0], np.cumsum(cnts[c])])
        pos = np.arange(n_e) - starts[key2]
        kb = key2 >> 1
        kp = key2 & 1
        cbase = scum[kb] + np.where(kp == 1, cpe[kb], 0)
        chunk = cbase + (pos >> 7)
        prow = pos & 127

        s_f32 = np.zeros((P, nch * P), dtype=np.float32)
        s_f32[prow, chunk * P + (dc2 & 127)] = 1.0
        sall = s_f32.astype(f8)

        idxflat = np.zeros(nch * P, dtype=np.int16)
        idxflat[chunk * P + prow] = (sc2 >> 1).astype(np.int16)
        idx16 = np.ascontiguousarray(
            np.tile(idxflat.reshape(nch * 8, 16).T, (8, 1))
        )

        # layer-1: edges grouped per block only (dst-sorted already)
        lo, hi = bounds[c], bounds[c + 1]
        sc1 = s_s[lo:hi]
        dc1 = (d_s[lo:hi] - c * SH).astype(np.int64)
        blk1 = dc1 >> 7
        starts1 = np.concatenate([[0], np.cumsum(cnts1[c])])
        pos1 = np.arange(hi - lo) - starts1[blk1]
        chunk1 = scum1[blk1] + (pos1 >> 7)
        prow1 = pos1 & 127

        s1_f32 = np.zeros((P, nch1 * P), dtype=np.float32)
        s1_f32[prow1, chunk1 * P + (dc1 & 127)] = 1.0
        sall1 = s1_f32.astype(f8)

        xg = np.zeros((nch1 * P, DIN), dtype=np.float32)
        xg[chunk1 * P + prow1, :] = xdi[sc1, :]
        xg = xg.astype(bf16).reshape(nch1, P, DIN)

        ids = c * SH + np.minimum(np.arange(NBLK * P), SH - 1)
        dcol = dinv[ids].reshape(NBLK, P).T.copy()  # [P, NBLK]
        per_core.append({"idx": idx16, "sall": sall, "sall1": sall1,
                         "xg": xg, "dcol": dcol})

    meta = {"cpe": cpe.tolist(), "cpo": cpo.tolist(),
            "scum": scum.tolist(), "nch": nch,
            "cpb1": cpb1.tolist(), "scum1": scum1.tolist(), "nch1": nch1}
    return per_core, dinv, meta


def _build_program(meta, with_bias):
    from concourse import bass, bacc, mybir
    import concourse.tile as tile
    from concourse import library_config

    f32 = mybir.dt.float32
    bf16 = mybir.dt.bfloat16
    f8 = mybir.dt.float8e4
    i16 = mybir.dt.int16
    AG = mybir.AluOpType
    ACT = mybir.ActivationFunctionType

    cpe, cpo, scum, nch = meta["cpe"], meta["cpo"], meta["scum"], meta["nch"]
    cpb1, scum1, nch1 = meta["cpb1"], meta["scum1"], meta["nch1"]
    CPT_MAX = max(cpe[b] + cpo[b] for b in range(NBLK))
    CPB1_MAX = max(cpb1)

    nc = bacc.Bacc("TRN2", target_bir_lowering=False, debug=False)

    xtsh = nc.declare_dram_parameter("xtsh", [P, SH], bf16, isOutput=False)
    xg = nc.declare_dram_parameter("xg", [nch1, P, DIN], bf16, isOutput=False)
    w1 = nc.declare_dram_parameter("w1", [P, DH], bf16, isOutput=False)
    w2f = nc.declare_dram_parameter("w2f", [P, 2 * DH], bf16, isOutput=False)
    w3f = nc.declare_dram_parameter("w3f", [P, 2 * DH], bf16, isOutput=False)
    idx = nc.declare_dram_parameter("idx", [P, nch * 8], i16, isOutput=False)
    sall = nc.declare_dram_parameter("sall", [P, nch * P], f8, isOutput=False)
    sall1 = nc.declare_dram_parameter("sall1", [P, nch1 * P], f8,
                                      isOutput=False)
    dcol = nc.declare_dram_parameter("dcol", [P, NBLK], f32, isOutput=False)
    id8 = nc.declare_dram_parameter("id8", [P, P], f8, isOutput=False)
    idb = nc.declare_dram_parameter("idb", [P, P], bf16, isOutput=False)
    if with_bias:
        badj = [nc.declare_dram_parameter(f"badj{l}", [SH, DH], f32,
                                          isOutput=False) for l in (1, 2, 3)]
    outp = nc.declare_dram_parameter("out", [SH, DH], f32, isOutput=True)

    ybin = {l: nc.dram_tensor(f"ybin{l}", [SH, DH], bf16) for l in (2, 3)}
    ybout = {l: nc.dram_tensor(f"ybout{l}", [N // 2, 2 * DH], bf16,
                               addr_space="Shared") for l in (2, 3)}

    with tile.TileContext(nc) as tc:
        with (
            tc.tile_pool(name="const", bufs=1) as cp_,
            tc.tile_pool(name="sb", bufs=3) as sb,
            tc.tile_pool(name="gp", bufs=2) as gp,
            tc.tile_pool(name="xgp", bufs=2) as xgp,
            tc.tile_pool(name="stp", bufs=2) as stp,
            tc.tile_pool(name="pp", bufs=2, space="PSUM") as pp,
            tc.tile_pool(name="pb", bufs=2, space="PSUM") as pbp,
            tc.tile_pool(name="tp", bufs=2, space="PSUM") as tpp,
        ):
            # library for dma_gather; must precede all gathers on GpSimd
            nc.gpsimd.load_library(library_config.mlp)

            w1sb = cp_.tile([P, DH], dtype=bf16)
            nc.sync.dma_start(out=w1sb[:], in_=w1[:, :])
            w2sb = cp_.tile([P, 2 * DH], dtype=bf16)
            nc.sync.dma_start(out=w2sb[:], in_=w2f[:, :])
            w3sb = cp_.tile([P, 2 * DH], dtype=bf16)
            nc.sync.dma_start(out=w3sb[:], in_=w3f[:, :])
            idxsb = cp_.tile([P, nch * 8], dtype=i16)
            nc.sync.dma_start(out=idxsb[:], in_=idx[:, :])
            dcolsb = cp_.tile([P, NBLK], dtype=f32)
            nc.sync.dma_start(out=dcolsb[:], in_=dcol[:, :])
            id8sb = cp_.tile([P, P], dtype=f8)
            nc.sync.dma_start(out=id8sb[:], in_=id8[:, :])
            idbsb = cp_.tile([P, P], dtype=bf16)
            nc.sync.dma_start(out=idbsb[:], in_=idb[:, :])
            xts1 = cp_.tile([P, SH], dtype=bf16)
            nc.sync.dma_start(out=xts1[:], in_=xtsh[:, :])
            xts = cp_.tile([P, 2 * SH], dtype=bf16)   # layers 2/3 lhsT
            yA = cp_.tile([P, NBLK * DH], dtype=bf16)  # own Y, layers 1/3
            yB = cp_.tile([P, NBLK * DH], dtype=bf16)  # own Y, layer 2

            # zero the gather-tile slots once: -1-padded gathers skip
            # writes, so unwritten lanes must hold finite values.
            for _ in range(2):
                gz = gp.tile([P, CPT_MAX, DH], dtype=bf16, tag="gt")
                nc.vector.memset(gz[:, :, :], 0.0)

            def phase1(w, lhs_slices, wsb, yown, yb):
                m = LASTM if w == NBLK - 1 else P
                ps = pp.tile([P, DH], dtype=f32, tag="p1ps")
                nk = len(lhs_slices)
                for k, lhs in enumerate(lhs_slices):
                    nc.tensor.matmul(out=ps[:m, :], lhsT=lhs,
                                     rhs=wsb[:, k * DH:(k + 1) * DH],
                                     start=(k == 0), stop=(k == nk - 1))
                nc.scalar.activation(out=yown[:m, w * DH:(w + 1) * DH],
                                     in_=ps[:m, :], func=ACT.Copy,
                                     scale=dcolsb[:m, w:w + 1])
                if yb is not None:
                    nc.sync.dma_start(out=yb[w * P:w * P + m, :],
                                      in_=yown[:m, w * DH:(w + 1) * DH])

            def allgather(l):
                nc.gpsimd.collective_compute(
                    "AllGather", AG.bypass,
                    replica_groups=[list(range(NC))],
                    ins=[ybin[l].ap().opt()],
                    outs=[ybout[l].ap().opt()])

            def epilogue(l, b, ps, yown_w, yb_w, wnext):
                """relu/scale + transposed stash + inline next phase1."""
                m = LASTM if b == NBLK - 1 else P
                if with_bias:
                    bj = sb.tile([P, DH], dtype=f32, tag="bj")
                    nc.sync.dma_start(out=bj[:m, :],
                                      in_=badj[l - 1][b * P:b * P + m, :])
                    nc.vector.tensor_tensor(out=ps[:m, :], in0=ps[:m, :],
                                            in1=bj[:m, :], op=AG.add)
                if l < 3:
                    xn = sb.tile([P, DH], dtype=bf16, tag="xn")
                    nc.scalar.activation(out=xn[:m, :], in_=ps[:m, :],
                                         func=ACT.Relu,
                                         scale=dcolsb[:m, b:b + 1])
                    for k in range(2):
                        tps = tpp.tile([P, P], dtype=bf16, tag="tps")
                        nc.tensor.transpose(
                            out=tps[:, :m],
                            in_=xn[:m, k * P:(k + 1) * P],
                            identity=idbsb[:m, :m])
                        nc.vector.tensor_copy(
                            out=xts[:, k * SH + b * P:k * SH + b * P + m],
                            in_=tps[:, :m])
                    phase1(b, [xts[:, b * P:b * P + m],
                               xts[:, SH + b * P:SH + b * P + m]],
                           wnext, yown_w, yb_w)
                else:
                    osb = sb.tile([P, DH], dtype=f32, tag="osb")
                    nc.scalar.activation(out=osb[:m, :], in_=ps[:m, :],
                                         func=ACT.Copy,
                                         scale=dcolsb[:m, b:b + 1])
                    nc.sync.dma_start(out=outp[b * P:b * P + m, :],
                                      in_=osb[:m, :])

            # ---- phase1 of layer 1: Y1 = dinv * (X @ W1), own shard ----
            for w in range(NBLK):
                m = LASTM if w == NBLK - 1 else P
                phase1(w, [xts1[:, w * P:w * P + m]], w1sb, yA, None)

            # ---- layer 1: gather-free scatter from host-staged Xg ----
            for b in range(NBLK):
                cp1 = cpb1[b]
                xgt = xgp.tile([P, CPB1_MAX, DIN], dtype=bf16, tag="xgt")
                src_ap = xg[scum1[b]:scum1[b] + cp1, :, :].transpose([1, 0, 2])
                nc.sync.dma_start(out=xgt[:, 0:cp1, :], in_=src_ap)
                st1 = stp.tile([P, CPB1_MAX * P], dtype=f8, tag="st1")
                nc.sync.dma_start(
                    out=st1[:, :cp1 * P],
                    in_=sall1[:, scum1[b] * P:(scum1[b] + cp1) * P])
                psB = pbp.tile([P, P], dtype=f32, tag="psB")
                for k in range(cp1):
                    nc.tensor.matmul(out=psB[:], lhsT=xgt[:, k, :],
                                     rhs=st1[:, k * P:(k + 1) * P],
                                     start=(k == 0), stop=(k == cp1 - 1))
                bsb = sb.tile([P, P], dtype=bf16, tag="bsb")
                nc.scalar.activation(out=bsb[:], in_=psB[:], func=ACT.Copy)
                ps = pp.tile([P, DH], dtype=f32, tag="agg")
                nc.tensor.matmul(out=ps[:], lhsT=bsb[:], rhs=w1sb[:],
                                 start=True, stop=False)
                nc.tensor.matmul(out=ps[:], lhsT=id8sb[:],
                                 rhs=yA[:, b * DH:(b + 1) * DH],
                                 start=False, stop=True)
                epilogue(1, b, ps, yB, ybin[2], w2sb)
            allgather(2)

            def scatter(l, table, yown_r, yown_w, yb_w, wnext):
                """Gather-based scatter layer l; inline phase1 of l+1."""
                for b in range(NBLK):
                    ce, co = cpe[b], cpo[b]
                    ct = ce + co
                    ic = scum[b] * 8
                    gt = gp.tile([P, CPT_MAX, DH], dtype=bf16, tag="gt")
                    nc.gpsimd.dma_gather(
                        gt[:, 0:ce, :], table[:, 0:DH],
                        idxsb[:, ic:ic + ce * 8],
                        ce * P, ce * P, DH, elem_step=2 * DH, queue_num=0)
                    nc.gpsimd.dma_gather(
                        gt[:, ce:ct, :], table[:, DH:2 * DH],
                        idxsb[:, ic + ce * 8:ic + ct * 8],
                        co * P, co * P, DH, elem_step=2 * DH, queue_num=0)
                    stt = stp.tile([P, CPT_MAX * P], dtype=f8, tag="stt")
                    nc.sync.dma_start(
                        out=stt[:, :ct * P],
                        in_=sall[:, scum[b] * P:(scum[b] + ct) * P])
                    ps = pp.tile([P, DH], dtype=f32, tag="agg")
                    for k in range(ct):
                        nc.tensor.matmul(out=ps[:],
                                         lhsT=stt[:, k * P:(k + 1) * P],
                                         rhs=gt[:, k, :],
                                         start=(k == 0), stop=False)
                    nc.tensor.matmul(out=ps[:], lhsT=id8sb[:],
                                     rhs=yown_r[:, b * DH:(b + 1) * DH],
                                     start=False, stop=True)
                    epilogue(l, b, ps, yown_w, yb_w, wnext)
                if l < 3:
                    allgather(l + 1)

            scatter(2, ybout[2], yB, yA, ybin[3], w3sb)
            scatter(3, ybout[3], yA, None, None, None)

    nc.compile()

    # sanity: library load must precede the first dma_gather on GpSimd
    pos_lib = pos_gather = None
    i = 0
    for blk in nc.m.functions[0].blocks:
        for inst in blk.instructions:
            tn = type(inst).__name__
            if pos_lib is None and tn == "InstPseudoReloadLibraryIndex":
                pos_lib = i
            if pos_gather is None and tn == "InstDMAGatherAnt":
                pos_gather = i
            i += 1
    assert pos_lib is not None, "library load missing"
    assert pos_gather is None or pos_lib < pos_gather, \
        f"library load at {pos_lib} after first gather at {pos_gather}"
    return nc


def kernel(x, edge_index, W1, b1, W2, b2, W3, b3, _trace=False):
    import ml_dtypes
    from concourse.bass_utils import run_bass_kernel_spmd

    bf16 = ml_dtypes.bfloat16
    f8 = ml_dtypes.float8_e4m3

    x = np.asarray(x, dtype=np.float32)
    b1 = np.asarray(b1, dtype=np.float32)
    b2 = np.asarray(b2, dtype=np.float32)
    b3 = np.asarray(b3, dtype=np.float32)
    with_bias = bool(np.any(b1) or np.any(b2) or np.any(b3))

    per_core, dinv, meta = _preprocess(x, edge_index)
    nc = _build_program(meta, with_bias)

    xt = np.ascontiguousarray(x.T)
    common = {
        "w1": np.asarray(W1, np.float32).astype(bf16),
        "w2f": np.ascontiguousarray(
            np.concatenate([np.asarray(W2, np.float32)[0:P, :],
                            np.asarray(W2, np.float32)[P:2 * P, :]], axis=1)
        ).astype(bf16),
        "w3f": np.ascontiguousarray(
            np.concatenate([np.asarray(W3, np.float32)[0:P, :],
                            np.asarray(W3, np.float32)[P:2 * P, :]], axis=1)
        ).astype(bf16),
        "id8": np.eye(P, dtype=np.float32).astype(f8),
        "idb": np.eye(P, dtype=np.float32).astype(bf16),
    }
    in_maps = []
    for c in range(NC):
        m = dict(common)
        m["xtsh"] = np.ascontiguousarray(
            xt[:, c * SH:(c + 1) * SH]).astype(bf16)
        m["idx"] = per_core[c]["idx"]
        m["sall"] = per_core[c]["sall"]
        m["sall1"] = per_core[c]["sall1"]
        m["xg"] = per_core[c]["xg"]
        m["dcol"] = per_core[c]["dcol"]
        if with_bias:
            dshard = dinv[c * SH:(c + 1) * SH]
            for l, b in ((1, b1), (2, b2), (3, b3)):
                m[f"badj{l}"] = np.ascontiguousarray(
                    b[None, :] / dshard[:, None]).astype(np.float32)
        in_maps.append(m)

    res = run_bass_kernel_spmd(nc, in_maps, list(range(NC)), trace=_trace)
    out = np.concatenate([res.results[c]["out"] for c in range(NC)], axis=0)
    if _trace:
        return out, res
    return out
